# revision 1
# baseline (speedup 1.0000x reference)
"""Bi-path windowed attention kernel for Trainium2 (8 NeuronCores).

Problem: x (4, 512, 128, 128) f32. Reference (per batch): raw-reshape to
tokens (128,128,512); global path = 2x2-window MHA (8 heads, hd=64) +
out-proj; local path = AvgPool2(x) -> raw-reshape tokens (64,64,512) ->
2x2-window MHA -> raw-reshape -> reflect-pad smoothing along y and x ->
bilinear x2 upsample; out = (global + local) transposed to (B, C, H, W).

Sharding: 8 shards = batch (4) x channel-half (2). Channel half
[256h, 256h+256) of x == token rows [64h, 64h+64), and the local path
for those rows only touches those channels, so shards are independent.
Each core: xs = x[b, 256h:256h+256] -> out[b, :, 64h:64h+64, :].

Layout: activations token-major, tokens grouped window-major
(partition = window (Ip, J); free = (di, dj, c)). Matmuls in float32r
(TF32-like; full PE rate at N=512) with stationary operands from PE
transposes. Attention is DVE/ACT/GPSIMD elementwise math over the free
dim (q.k scores in bf16, rest f32). Local-path pooling / smoothing /
upsample run channel-major, exchanging with token-major stages through
DRAM scratch.
"""
import sys
if '/opt/trn_rl_repo' not in sys.path:
    sys.path.append('/opt/trn_rl_repo')  # fallback; axon sitecustomize copy wins
import numpy as np

_RUN_CACHE = {}

B, C, H, W = 4, 512, 128, 128
NH, HD = 8, 64


def _mk_tile_context_fixed():
    import concourse.mybir as mybir
    import concourse.tile as tile
    from concourse.vector_clock import ScopedClock, VectorClock

    class TileContextFixed(tile.TileContext):
        """Works around a walrus codegen limit in this toolchain: max ONE
        sync-wait per instruction. Extra waits are peeled onto single-wait
        NoOps on the same engine; the kernel-tail drain gets per-proc
        single-wait NOPs instead of one multi-wait drain."""
        _ctr = 0

        def _lower_ordered_insts(self, ordered):
            cls = type(self)
            for bb_name, insts in ordered.items():
                new_list = []
                for inst in insts:
                    try:
                        si = inst.sync_info
                    except Exception:
                        si = None
                    if si is not None and len(si.on_wait) > 1:
                        waits = list(si.on_wait)
                        extra, keep = waits[:-1], waits[-1:]
                        for w in extra:
                            nop = mybir.InstNoOp(
                                name=f"I-waitsplit-{cls._ctr}", ins=[], outs=[])
                            cls._ctr += 1
                            nop.engine = inst.engine
                            nop.sync_info = mybir.SyncInfo(
                                on_wait=[w], on_update=[])
                            self.nc.register_instruction(nop, overwrite=True)
                            new_list.append(nop)
                        inst.sync_info = mybir.SyncInfo(
                            on_wait=keep, on_update=list(si.on_update))
                    new_list.append(inst)
                ordered[bb_name] = new_list
            super()._lower_ordered_insts(ordered)

        def _drain_and_barrier(self, tick_clock, wait_clock):
            gc = tick_clock.global_clock
            scoped = gc if hasattr(gc, 'items') else ScopedClock({None: gc})
            for scope, vc in scoped.items():
                n = len(vc)
                for proc in range(n):
                    t = vc[proc]
                    if t <= 0:
                        continue
                    vec = [0] * n
                    vec[proc] = t
                    nop = self.nc.sync.nop()
                    wait_clock.add_sem_waits(
                        nop.ins, ScopedClock({scope: VectorClock(vec)}))
            self.nc.sync.drain()
            self.nc.all_engine_barrier()
            popped = self.nc._tile_sem_poison_stack.pop()
            assert popped is self._sem_poison
            self.nc.clear_and_free_semaphores(
                list(self.sems.allocated().values()))
            self.nc.all_engine_barrier()

    return TileContextFixed


def _dap(handle, off, dims):
    """Raw DRAM access pattern: flat element offset + [step, count] dims."""
    import concourse.bass as bass
    base = handle[:]
    return bass.AP(tensor=base.tensor, offset=base.offset + off,
                   ap=[list(d) for d in dims])


def _sap(tile_, off, dims):
    """SBUF tile sub-AP: keep partition dim, replace free dims."""
    import concourse.bass as bass
    base = tile_[:]
    return bass.AP(tensor=base.tensor, offset=base.offset + off,
                   ap=[list(base.ap[0])] + [list(d) for d in dims])


def _build_nc():
    import concourse.bass as bass
    import concourse.mybir as mybir
    from concourse.masks import make_identity
    TileContextFixed = _mk_tile_context_fixed()
    f32 = mybir.dt.float32
    f32r = mybir.dt.float32r
    bf16 = mybir.dt.bfloat16
    Copy = mybir.ActivationFunctionType.Copy
    Exp = mybir.ActivationFunctionType.Exp
    ADD = mybir.AluOpType.add
    MUL = mybir.AluOpType.mult
    AX = mybir.AxisListType.X
    THIRD = 1.0 / 3.0

    nc = bass.Bass()
    xs = nc.declare_dram_parameter("xs", [256, H, W], f32, isOutput=False)
    wqkv = nc.declare_dram_parameter("Wqkv", [C, 3 * C], f32, isOutput=False)
    bqkv = nc.declare_dram_parameter("bqkv", [3 * C], f32, isOutput=False)
    wproj = nc.declare_dram_parameter("Wproj", [C, C], f32, isOutput=False)
    bproj = nc.declare_dram_parameter("bproj", [C], f32, isOutput=False)
    out = nc.declare_dram_parameter("out", [C, 64, W], f32, isOutput=True)

    pooled = nc.dram_tensor("pooled", [256 * 64 * 64], f32)
    lout = nc.dram_tensor("lout", [2048 * 512], f32)
    lup = nc.dram_tensor("lup", [256 * H * W], f32)

    with TileContextFixed(nc) as tc:
        with (
            tc.tile_pool(name="consts", bufs=1) as consts,
            tc.tile_pool(name="work", bufs=2) as work,
            tc.tile_pool(name="psum", bufs=2, space="PSUM") as psum,
            tc.tile_pool(name="psumT", bufs=4, space="PSUM") as psumT,
        ):
            # ---- constants ----
            ident = consts.tile([128, 128], f32)
            make_identity(nc, ident[:])
            wqkv_r = consts.tile([128, 4, 1536], f32r)
            wproj_r = consts.tile([128, 4, 512], f32r)
            for kc in range(4):
                wst = work.tile([128, 1536], f32, tag="xwm")
                nc.sync.dma_start(out=wst, in_=_dap(
                    wqkv, kc * 128 * 1536, [[1536, 128], [1, 1536]]))
                nc.vector.tensor_copy(wqkv_r[:, kc, :], wst[:])
                wst2 = work.tile([128, 512], f32, tag="lupg")
                nc.sync.dma_start(out=wst2, in_=_dap(
                    wproj, kc * 128 * 512, [[512, 128], [1, 512]]))
                nc.vector.tensor_copy(wproj_r[:, kc, :], wst2[:])
            bqkv_b = consts.tile([128, 1536], f32)
            nc.sync.dma_start(out=bqkv_b, in_=_dap(bqkv, 0, [[0, 128], [1, 1536]]))
            bproj_b = consts.tile([128, 512], f32)
            nc.sync.dma_start(out=bproj_b, in_=_dap(bproj, 0, [[0, 128], [1, 512]]))

            # ---- stage B: avg-pool 2x2 channel-major -> pooled scratch ----
            # strips of 16 input rows -> 8 pooled rows each
            for cc in range(2):
                for yt in range(8):
                    pin = work.tile([128, 2048], f32, tag="xwm")
                    nc.sync.dma_start(out=pin, in_=_dap(
                        xs, cc * 128 * 16384 + yt * 16 * 128,
                        [[16384, 128], [1, 2048]]))
                    t1 = work.tile([128, 512], f32, tag="lupg")
                    nc.vector.tensor_add(
                        t1[:], _sap(pin, 0, [[256, 8], [2, 64]]),
                        _sap(pin, 1, [[256, 8], [2, 64]]))
                    t2 = work.tile([128, 512], f32, tag="g1")
                    nc.vector.tensor_add(
                        t2[:], _sap(pin, 128, [[256, 8], [2, 64]]),
                        _sap(pin, 129, [[256, 8], [2, 64]]))
                    t3 = work.tile([128, 512], f32, tag="t3")
                    nc.gpsimd.tensor_add(t3[:], t1[:], t2[:])
                    t4 = work.tile([128, 512], f32, tag="outt")
                    nc.scalar.activation(t4[:], t3[:], Copy, scale=0.25)
                    nc.sync.dma_start(
                        out=_dap(pooled, cc * 128 * 4096 + yt * 512,
                                 [[4096, 128], [1, 512]]),
                        in_=t4)

            # ---- shared qkv + attention emitter (window-major tile) ----
            def qkv_attn_tile(x_wm):
                """x_wm: [128 win, 2048] = (di, dj, c). Returns O [128,2048] f32."""
                xt = work.tile([128, 4, 4, 128], f32r, tag="xt")
                for i in range(4):
                    for kc in range(4):
                        psT = psumT.tile([128, 128], f32, tag="psT")
                        nc.tensor.transpose(
                            psT[:], _sap(x_wm, i * 512 + kc * 128, [[1, 128]]),
                            ident[:])
                        nc.scalar.copy(out=xt[:, i, kc, :], in_=psT[:])
                qk = work.tile([128, 4, 1024], bf16, tag="qk")
                vt = work.tile([128, 4, 512], f32, tag="vt")
                for i in range(4):
                    for nb in range(3):
                        psQ = psum.tile([128, 512], f32, tag="psQ")
                        for kc in range(4):
                            nc.tensor.matmul(
                                psQ[:], xt[:, i, kc, :],
                                wqkv_r[:, kc, nb * 512:(nb + 1) * 512],
                                start=(kc == 0), stop=(kc == 3))
                        dst = (qk[:, i, 0:512] if nb == 0 else
                               qk[:, i, 512:1024] if nb == 1 else vt[:, i, :])
                        nc.vector.tensor_add(
                            dst, psQ[:], bqkv_b[:, nb * 512:(nb + 1) * 512])
                # scores S[w,i,n,j] = sum_d q[i,n,d]*k[j,n,d]  (bf16 in, f32 out)
                S = work.tile([128, 128], f32, tag="S")
                tmpb = work.tile([128, 2048], bf16, tag="tmpb")
                tmpb2 = work.tile([128, 2048], bf16, tag="lupg")
                for j in range(4):
                    k_j = _sap(qk, j * 1024 + 512, [[0, 4], [64, 8], [1, 64]])
                    q_all = _sap(qk, 0, [[1024, 4], [64, 8], [1, 64]])
                    dst = tmpb if j % 2 == 0 else tmpb2
                    eng = nc.vector if j % 2 == 0 else nc.gpsimd
                    eng.tensor_mul(dst[:], q_all, k_j)
                    nc.vector.tensor_reduce(
                        out=_sap(S, j, [[32, 4], [4, 8]]),
                        in_=_sap(dst, 0, [[512, 4], [64, 8], [1, 64]]),
                        axis=AX, op=ADD)
                E = work.tile([128, 128], f32, tag="E")
                nc.scalar.activation(E[:], S[:], Exp, scale=float(HD) ** -0.5)
                D = work.tile([128, 32], f32, tag="D")
                nc.vector.tensor_reduce(
                    out=D[:], in_=_sap(E, 0, [[4, 32], [1, 4]]), axis=AX, op=ADD)
                R = work.tile([128, 32], f32, tag="R")
                nc.vector.reciprocal(R[:], D[:])
                P = work.tile([128, 128], f32, tag="P")
                nc.vector.tensor_mul(P[:], E[:], _sap(R, 0, [[1, 32], [0, 4]]))
                # O[w,i,n,d] = sum_j P[w,i,n,j] * v[w,j,n,d]
                # muls split DVE/Pool, tree-style adds to shorten the chain
                O = work.tile([128, 2048], f32, tag="O")
                tmpO = work.tile([128, 2048], f32, tag="tmpO")
                tmpO2 = work.tile([128, 2048], f32, tag="sums")
                pv = [(_sap(P, j, [[32, 4], [4, 8], [0, 64]]),
                       _sap(vt, j * 512, [[0, 4], [64, 8], [1, 64]]))
                      for j in range(4)]
                nc.vector.tensor_mul(O[:], pv[0][0], pv[0][1])
                nc.vector.tensor_mul(tmpO[:], pv[1][0], pv[1][1])
                nc.gpsimd.tensor_mul(tmpO2[:], pv[2][0], pv[2][1])
                nc.gpsimd.tensor_add(O[:], O[:], tmpO[:])
                nc.vector.tensor_mul(tmpO[:], pv[3][0], pv[3][1])
                nc.gpsimd.tensor_add(tmpO2[:], tmpO2[:], tmpO[:])
                nc.gpsimd.tensor_add(O[:], O[:], tmpO2[:])
                return O

            # ---- stage C: local attention (4 ltiles) -> lout scratch ----
            for lt in range(4):
                x_l = work.tile([128, 2, 1024], f32, tag="xwm")
                for di in range(2):
                    nc.sync.dma_start(out=x_l[:, di, :], in_=_dap(
                        pooled, lt * 262144 + di * 32768,
                        [[65536, 4], [1024, 32], [1, 1024]]))
                O_l = qkv_attn_tile(x_l)
                for di in range(2):
                    nc.sync.dma_start(
                        out=_dap(lout, lt * 262144 + di * 32768,
                                 [[65536, 4], [1024, 32], [1, 1024]]),
                        in_=_sap(O_l, di * 1024, [[1, 1024]]))

            # ---- stage D: smoothing + bilinear x2 upsample -> lup scratch ----
            # All scale factors folded algebraically: with raw sums
            #   a1[y] = l[y-1]+l[y] (reflect y=0), a2[y] = l[y]+l[y+1] (reflect
            #   x=63), sraw = a1+a2 (= 2*(lx+ly))
            #   u'[2y] = sraw[y] + sraw[y-1]/3, u'[2y+1] = sraw[y] + sraw[y+1]/3
            #   L'[2x] = u'[x] + u'[x-1]/3,  L'[2x+1] = u'[x] + u'[x+1]/3
            #   lup = 0.28125 * L'   (= 0.5 * 0.375 * 0.75 * ... collapsed)
            # borders use clamped taps; the stt form works there unchanged.
            for cc in range(2):
                for st in range(4):          # strips of 16 pooled rows
                    y0 = st * 16
                    r0, r1 = max(y0 - 2, 0), min(y0 + 17, 64)   # Lp rows
                    s0, s1 = max(y0 - 1, 0), min(y0 + 17, 64)   # sraw rows
                    nlr = r1 - r0
                    nsr = s1 - s0
                    Lp = work.tile([128, nlr * 64], f32, tag="xwm")
                    nc.sync.dma_start(out=Lp, in_=_dap(
                        lout, cc * 128 * 4096 + r0 * 64,
                        [[4096, 128], [1, nlr * 64]]))

                    def lrow(y):  # strip-local Lp row offset
                        return (y - r0) * 64

                    def srow(y):  # strip-local sraw row offset
                        return (y - s0) * 64

                    a1 = work.tile([128, nsr * 64], f32, tag="tmpO")
                    ym = max(s0, 1)  # main region rows [ym, s1)
                    nc.vector.tensor_add(
                        _sap(a1, srow(ym), [[1, (s1 - ym) * 64]]),
                        _sap(Lp, lrow(ym - 1), [[1, (s1 - ym) * 64]]),
                        _sap(Lp, lrow(ym), [[1, (s1 - ym) * 64]]))
                    if s0 == 0:  # reflect top: a1[0] = l[0] + l[1]
                        nc.vector.tensor_add(
                            _sap(a1, 0, [[1, 64]]),
                            _sap(Lp, 0, [[1, 64]]),
                            _sap(Lp, 64, [[1, 64]]))
                    a2 = work.tile([128, nsr * 64], f32, tag="O")
                    nc.gpsimd.tensor_add(
                        _sap(a2, 0, [[64, nsr], [1, 63]]),
                        _sap(Lp, lrow(s0), [[64, nsr], [1, 63]]),
                        _sap(Lp, lrow(s0) + 1, [[64, nsr], [1, 63]]))
                    nc.gpsimd.tensor_add(
                        _sap(a2, 63, [[64, nsr]]),
                        _sap(Lp, lrow(s0) + 63, [[64, nsr]]),
                        _sap(Lp, lrow(s0) + 62, [[64, nsr]]))
                    sraw = work.tile([128, nsr * 64], f32, tag="xwm")
                    nc.vector.tensor_add(sraw[:], a1[:], a2[:])
                    # y-upsample (u' rows Y-2*y0, 32 rows x 64 cols)
                    u = work.tile([128, 2048], f32, tag="vt")
                    ye = max(y0, 1)  # even rows needing y-1
                    nc.vector.scalar_tensor_tensor(
                        out=_sap(u, (ye - y0) * 128, [[128, y0 + 16 - ye], [1, 64]]),
                        in0=_sap(sraw, srow(ye - 1), [[64, y0 + 16 - ye], [1, 64]]),
                        scalar=THIRD,
                        in1=_sap(sraw, srow(ye), [[64, y0 + 16 - ye], [1, 64]]),
                        op0=MUL, op1=ADD)
                    if y0 == 0:  # Y=0: taps both row 0
                        nc.vector.scalar_tensor_tensor(
                            out=_sap(u, 0, [[1, 64]]),
                            in0=_sap(sraw, 0, [[1, 64]]), scalar=THIRD,
                            in1=_sap(sraw, 0, [[1, 64]]), op0=MUL, op1=ADD)
                    yo1 = min(y0 + 16, 63)  # odd rows needing y+1: y in [y0, yo1)
                    nc.vector.scalar_tensor_tensor(
                        out=_sap(u, 64, [[128, yo1 - y0], [1, 64]]),
                        in0=_sap(sraw, srow(y0 + 1), [[64, yo1 - y0], [1, 64]]),
                        scalar=THIRD,
                        in1=_sap(sraw, srow(y0), [[64, yo1 - y0], [1, 64]]),
                        op0=MUL, op1=ADD)
                    if y0 + 16 == 64:  # Y=127: taps both row 63
                        nc.vector.scalar_tensor_tensor(
                            out=_sap(u, 31 * 64, [[1, 64]]),
                            in0=_sap(sraw, srow(63), [[1, 64]]), scalar=THIRD,
                            in1=_sap(sraw, srow(63), [[1, 64]]), op0=MUL, op1=ADD)
                    # x-upsample per 16-row half + final scale + store
                    for hf in range(2):
                        Lh = work.tile([128, 2048], f32, tag="O")
                        ub = hf * 16 * 64  # u offset of this half's rows
                        nc.vector.scalar_tensor_tensor(
                            out=_sap(Lh, 2, [[128, 16], [2, 63]]),
                            in0=_sap(u, ub, [[64, 16], [1, 63]]), scalar=THIRD,
                            in1=_sap(u, ub + 1, [[64, 16], [1, 63]]),
                            op0=MUL, op1=ADD)
                        nc.vector.scalar_tensor_tensor(
                            out=_sap(Lh, 0, [[128, 16]]),
                            in0=_sap(u, ub, [[64, 16]]), scalar=THIRD,
                            in1=_sap(u, ub, [[64, 16]]), op0=MUL, op1=ADD)
                        nc.vector.scalar_tensor_tensor(
                            out=_sap(Lh, 1, [[128, 16], [2, 63]]),
                            in0=_sap(u, ub + 1, [[64, 16], [1, 63]]), scalar=THIRD,
                            in1=_sap(u, ub, [[64, 16], [1, 63]]),
                            op0=MUL, op1=ADD)
                        nc.vector.scalar_tensor_tensor(
                            out=_sap(Lh, 127, [[128, 16]]),
                            in0=_sap(u, ub + 63, [[64, 16]]), scalar=THIRD,
                            in1=_sap(u, ub + 63, [[64, 16]]), op0=MUL, op1=ADD)
                        Lsc = work.tile([128, 2048], f32, tag="tmpO")
                        nc.scalar.activation(Lsc[:], Lh[:], Copy, scale=0.28125)
                        nc.sync.dma_start(
                            out=_dap(lup,
                                     cc * 128 * 16384 + (2 * y0 + 16 * hf) * 128,
                                     [[16384, 128], [1, 2048]]),
                            in_=Lsc)

            # ---- stage A: global path (16 wtiles) ----
            for ti in range(16):
                x_wm = work.tile([128, 2, 1024], f32, tag="xwmA")
                for di in range(2):
                    nc.sync.dma_start(out=x_wm[:, di, :], in_=_dap(
                        xs, ti * 4 * 65536 + di * 65536,
                        [[131072, 2], [1024, 64], [1, 1024]]))
                O = qkv_attn_tile(x_wm)
                sums = work.tile([128, 4, 512], f32, tag="sums")
                for i in range(4):
                    at = work.tile([128, 4, 128], f32r, tag="at")
                    for kc in range(4):
                        psT2 = psumT.tile([128, 128], f32, tag="psT")
                        nc.tensor.transpose(
                            psT2[:], _sap(O, i * 512 + kc * 128, [[1, 128]]),
                            ident[:])
                        nc.scalar.copy(out=at[:, kc, :], in_=psT2[:])
                    psP = psum.tile([128, 512], f32, tag="psQ")
                    for kc in range(4):
                        nc.tensor.matmul(psP[:], at[:, kc, :], wproj_r[:, kc, :],
                                         start=(kc == 0), stop=(kc == 3))
                    di, dj = i >> 1, i & 1
                    lupg = work.tile([128, 512], f32, tag="lupg")
                    nc.sync.dma_start(out=lupg, in_=_dap(
                        lup, ti * 262144 + di * 65536 + dj * 512,
                        [[131072, 2], [1024, 64], [1, 512]]))
                    g1 = work.tile([128, 512], f32, tag="g1")
                    nc.vector.tensor_add(g1[:], psP[:], bproj_b[:])
                    nc.gpsimd.tensor_add(sums[:, i, :], g1[:], lupg[:])
                # final transpose to (C, h, w) + DMA out
                for ch in range(4):
                    outt = work.tile([128, 512], f32, tag="outt")
                    for i in range(4):
                        di, dj = i >> 1, i & 1
                        psF = psumT.tile([128, 128], f32, tag="psT")
                        nc.tensor.transpose(
                            psF[:], _sap(sums, i * 512 + ch * 128, [[1, 128]]),
                            ident[:])
                        nc.scalar.copy(
                            out=_sap(outt, di * 128 + dj, [[256, 2], [2, 64]]),
                            in_=_sap(psF, 0, [[64, 2], [1, 64]]))
                    nc.sync.dma_start(
                        out=_dap(out, ch * 128 * 8192 + ti * 512,
                                 [[8192, 128], [128, 4], [1, 128]]),
                        in_=outt)
    return nc


def _get_nc():
    if 'nc' not in _RUN_CACHE:
        _RUN_CACHE['nc'] = _build_nc()
    return _RUN_CACHE['nc']


def make_in_maps(inputs):
    x = np.ascontiguousarray(np.asarray(inputs['x'], dtype=np.float32))
    Wqkv = np.ascontiguousarray(np.asarray(inputs['Wqkv'], dtype=np.float32))
    bqkv = np.ascontiguousarray(np.asarray(inputs['bqkv'], dtype=np.float32))
    Wproj = np.ascontiguousarray(np.asarray(inputs['Wproj'], dtype=np.float32))
    bproj = np.ascontiguousarray(np.asarray(inputs['bproj'], dtype=np.float32))
    in_maps = []
    shards = []
    for b in range(B):
        for half in range(2):
            shards.append((b, half))
            in_maps.append({
                "xs": np.ascontiguousarray(x[b, 256 * half:256 * (half + 1)]),
                "Wqkv": Wqkv, "bqkv": bqkv, "Wproj": Wproj, "bproj": bproj,
            })
    return in_maps, shards


def kernel(**inputs):
    from concourse.bass_utils import run_bass_kernel_spmd
    nc = _get_nc()
    in_maps, shards = make_in_maps(inputs)
    r = run_bass_kernel_spmd(nc, in_maps, core_ids=list(range(8)))
    _RUN_CACHE['last_result'] = r
    full = np.empty((B, C, H, W), dtype=np.float32)
    for (b, half), res in zip(shards, r.results):
        full[b, :, 64 * half:64 * (half + 1), :] = res["out"]
    return full



# revision 7
# speedup vs baseline: 1.0868x; 1.0868x over previous
"""Bi-path windowed attention kernel for Trainium2 (8 NeuronCores), v2.

Problem: x (4, 512, 128, 128) f32. Reference (per batch): raw-reshape to
tokens (128,128,512); global path = 2x2-window MHA (8 heads, hd=64) +
out-proj; local path = AvgPool2(x) -> raw-reshape tokens (64,64,512) ->
2x2-window MHA -> raw-reshape -> reflect-pad smoothing -> bilinear x2
upsample; out = (global + local) transposed to (B, C, H, W).

Sharding: 8 shards = batch (4) x channel-half (2); channel half h of x ==
token rows [64h, 64h+64), and both paths for those rows stay inside the
shard, so shards are independent.

v2 design (vs v1 elementwise attention): tokens-on-partitions, all-bf16
matmul pipeline with the attention itself on the PE:
 - tile = 32 windows x 4 tokens = 128 partitions (p = di*64 + 2w + dj)
 - Q^T/K^T computed directly in [head-dim, token] layout (lhsT = Wq/Wk
   chunks, rhs = x^T chunks); V token-major.
 - scores: per head one [64]-contraction matmul giving all 128x128 token
   pairs of the 32-window group, plus a rank-32 accumulation matmul that
   adds +C to same-window pairs (uniform boost cancels in softmax; the
   off-window pairs stay ~exp(-C/8) smaller = masked).
 - softmax without max-subtraction; q-bias cancels in softmax, k-bias
   becomes a per-token factor ev = exp(scale*K~_h.bq_h) folded into V and
   into a 65th "ones" column that makes the P.V matmul also emit the
   softmax denominator. v-bias: global path folds bv@Wproj+bproj into an
   output bias added via the smoothing stage; local path adds bv to V.
 - local pooling is 4 contiguous-token adds in token-major layout (the
   raw reshape makes pooled-token gathers contiguous in DRAM).
 - layout exchanges (token-major <-> channel-major) are free flat
   reinterpretations of DRAM scratch.
"""
import sys
if '/opt/trn_rl_repo' not in sys.path:
    sys.path.append('/opt/trn_rl_repo')
import numpy as np

_RUN_CACHE = {}

B, C, H, W = 4, 512, 128, 128
NH, HD = 8, 64
SCALE = float(HD) ** -0.5
MSQ = 11.3125   # bf16(sqrt(128)); MSQ^2 ~ 128 uniform in-window boost


def _mk_tile_context_fixed():
    import concourse.mybir as mybir
    import concourse.tile as tile
    from concourse.vector_clock import ScopedClock, VectorClock

    class TileContextFixed(tile.TileContext):
        """Works around a walrus codegen limit in this toolchain: max ONE
        sync-wait per instruction. Extra waits are peeled onto single-wait
        NoOps on the same engine; the kernel-tail drain gets per-proc
        single-wait NOPs instead of one multi-wait drain."""
        _ctr = 0

        def _lower_ordered_insts(self, ordered):
            cls = type(self)
            for bb_name, insts in ordered.items():
                new_list = []
                for inst in insts:
                    try:
                        si = inst.sync_info
                    except Exception:
                        si = None
                    if si is not None and len(si.on_wait) > 1:
                        waits = list(si.on_wait)
                        extra, keep = waits[:-1], waits[-1:]
                        for w in extra:
                            nop = mybir.InstNoOp(
                                name=f"I-waitsplit-{cls._ctr}", ins=[], outs=[])
                            cls._ctr += 1
                            nop.engine = inst.engine
                            nop.sync_info = mybir.SyncInfo(
                                on_wait=[w], on_update=[])
                            self.nc.register_instruction(nop, overwrite=True)
                            new_list.append(nop)
                        inst.sync_info = mybir.SyncInfo(
                            on_wait=keep, on_update=list(si.on_update))
                    new_list.append(inst)
                ordered[bb_name] = new_list
            super()._lower_ordered_insts(ordered)

        def _drain_and_barrier(self, tick_clock, wait_clock):
            gc = tick_clock.global_clock
            scoped = gc if hasattr(gc, 'items') else ScopedClock({None: gc})
            for scope, vc in scoped.items():
                n = len(vc)
                for proc in range(n):
                    t = vc[proc]
                    if t <= 0:
                        continue
                    vec = [0] * n
                    vec[proc] = t
                    nop = self.nc.sync.nop()
                    wait_clock.add_sem_waits(
                        nop.ins, ScopedClock({scope: VectorClock(vec)}))
            self.nc.sync.drain()
            self.nc.all_engine_barrier()
            popped = self.nc._tile_sem_poison_stack.pop()
            assert popped is self._sem_poison
            self.nc.clear_and_free_semaphores(
                list(self.sems.allocated().values()))
            self.nc.all_engine_barrier()

    return TileContextFixed


def _dap(handle, off, dims):
    """Raw DRAM access pattern: flat element offset + [step, count] dims."""
    import concourse.bass as bass
    base = handle[:]
    return bass.AP(tensor=base.tensor, offset=base.offset + off,
                   ap=[list(d) for d in dims])


def _sap(tile_, off, dims):
    """SBUF tile sub-AP: keep partition dim, replace free dims."""
    import concourse.bass as bass
    base = tile_[:]
    return bass.AP(tensor=base.tensor, offset=base.offset + off,
                   ap=[list(base.ap[0])] + [list(d) for d in dims])


def _build_nc():
    import concourse.bass as bass
    import concourse.mybir as mybir
    from concourse.masks import make_identity
    TileContextFixed = _mk_tile_context_fixed()
    f32 = mybir.dt.float32
    bf = mybir.dt.bfloat16
    Copy = mybir.ActivationFunctionType.Copy
    Exp = mybir.ActivationFunctionType.Exp
    ADD = mybir.AluOpType.add
    MUL = mybir.AluOpType.mult
    THIRD = 1.0 / 3.0

    nc = bass.Bass()
    xs = nc.declare_dram_parameter("xs", [8192 * 512], bf, isOutput=False)
    wg_d = nc.declare_dram_parameter("wqkv_g", [C * 3 * C], bf, isOutput=False)
    wl_d = nc.declare_dram_parameter("wqkv_l", [C * 3 * C], bf, isOutput=False)
    wp_d = nc.declare_dram_parameter("wproj", [C * C], bf, isOutput=False)
    bq_d = nc.declare_dram_parameter("bqmat", [C * 8], bf, isOutput=False)
    mb_d = nc.declare_dram_parameter("maskb", [32 * 128], bf, isOutput=False)
    ob_d = nc.declare_dram_parameter("obias", [C], bf, isOutput=False)
    bv_d = nc.declare_dram_parameter("bvv", [C], bf, isOutput=False)
    out = nc.declare_dram_parameter("out", [C, 64, W], f32, isOutput=True)

    lout = nc.dram_tensor("lout", [2048 * 512], bf)
    lup = nc.dram_tensor("lup", [8192 * 512], bf)

    with TileContextFixed(nc) as tc:
        with (
            tc.tile_pool(name="consts", bufs=1) as consts,
            tc.tile_pool(name="work", bufs=2) as work,
            tc.tile_pool(name="ps2", bufs=2, space="PSUM") as ps2,
            tc.tile_pool(name="ps1", bufs=1, space="PSUM") as ps1,
        ):
            # ---- constants ----
            identb = consts.tile([128, 128], bf)
            make_identity(nc, identb[:])
            identf = consts.tile([128, 128], f32)
            make_identity(nc, identf[:])
            wg = consts.tile([128, 4, 1536], bf)
            nc.sync.dma_start(out=wg, in_=_dap(
                wg_d, 0, [[1536, 128], [196608, 4], [1, 1536]]))
            wl = consts.tile([128, 4, 1536], bf)
            nc.sync.dma_start(out=wl, in_=_dap(
                wl_d, 0, [[1536, 128], [196608, 4], [1, 1536]]))
            wpj = consts.tile([128, 4, 512], bf)
            nc.sync.dma_start(out=wpj, in_=_dap(
                wp_d, 0, [[512, 128], [65536, 4], [1, 512]]))
            bqb = consts.tile([128, 4, 8], bf)
            nc.sync.dma_start(out=bqb, in_=_dap(
                bq_d, 0, [[8, 128], [1024, 4], [1, 8]]))
            mbt = consts.tile([32, 128], bf)
            nc.sync.dma_start(out=mbt, in_=_dap(mb_d, 0, [[128, 32], [1, 128]]))
            obb = consts.tile([128, 512], bf)
            nc.sync.dma_start(out=obb, in_=_dap(ob_d, 0, [[0, 128], [1, 512]]))
            bvb = consts.tile([128, 512], bf)
            nc.sync.dma_start(out=bvb, in_=_dap(bv_d, 0, [[0, 128], [1, 512]]))

            # ---- shared attention emitter ----
            # x_t: [128 tok, 512] bf16 (p = di*64 + 2w + dj). Returns O
            # [128, 512] bf16 token-major.
            def emit_attn(x_t, wt, add_bv):
                psT = ps2.tile([128, 512], bf, tag="tr")
                for kc in range(4):
                    nc.tensor.transpose(
                        psT[:, kc * 128:(kc + 1) * 128],
                        x_t[:, kc * 128:(kc + 1) * 128], identb[:])
                xt = work.tile([128, 4, 128], bf, tag="xt")
                nc.scalar.copy(out=_sap(xt, 0, [[1, 512]]), in_=psT[:])
                # QT/KT: [hd, tok] per 128-chunk of (h,d)
                psQT = ps2.tile([128, 512], f32, tag="mm")
                for hc in range(4):
                    for kc in range(4):
                        nc.tensor.matmul(
                            psQT[:, hc * 128:(hc + 1) * 128],
                            wt[:, kc, hc * 128:(hc + 1) * 128],
                            xt[:, kc, :], start=(kc == 0), stop=(kc == 3))
                qt = work.tile([128, 4, 128], bf, tag="qt")
                nc.scalar.copy(out=_sap(qt, 0, [[1, 512]]), in_=psQT[:])
                psKT = ps2.tile([128, 512], f32, tag="mm")
                for hc in range(4):
                    for kc in range(4):
                        nc.tensor.matmul(
                            psKT[:, hc * 128:(hc + 1) * 128],
                            wt[:, kc, 512 + hc * 128:512 + (hc + 1) * 128],
                            xt[:, kc, :], start=(kc == 0), stop=(kc == 3))
                kt = work.tile([128, 4, 128], bf, tag="kt")
                nc.scalar.copy(out=_sap(kt, 0, [[1, 512]]), in_=psKT[:])
                # kb[tok, h] = K~_h . bq_h
                psKB = ps1.tile([128, 8], f32, tag="kb")
                for hc in range(4):
                    nc.tensor.matmul(psKB[:], kt[:, hc, :], bqb[:, hc, :],
                                     start=(hc == 0), stop=(hc == 3))
                ev = work.tile([128, 8], f32, tag="ev")
                nc.scalar.activation(ev[:], psKB[:], Exp, scale=SCALE)
                # V (after QT/KT so the "mm" rotation can't deadlock)
                psV = ps2.tile([128, 512], f32, tag="mm")
                for kc in range(4):
                    nc.tensor.matmul(psV[:], xt[:, kc, :],
                                     wt[:, kc, 1024:1536],
                                     start=(kc == 0), stop=(kc == 3))
                vs = work.tile([128, 8, 65], bf, tag="vs")
                if add_bv:
                    vb = work.tile([128, 512], f32, tag="vb")
                    nc.vector.tensor_add(vb[:], psV[:], bvb[:])
                    nc.vector.tensor_mul(
                        _sap(vs, 0, [[65, 8], [1, 64]]),
                        _sap(vb, 0, [[64, 8], [1, 64]]),
                        _sap(ev, 0, [[1, 8], [0, 64]]))
                else:
                    nc.vector.tensor_mul(
                        _sap(vs, 0, [[65, 8], [1, 64]]),
                        _sap(psV, 0, [[64, 8], [1, 64]]),
                        _sap(ev, 0, [[1, 8], [0, 64]]))
                nc.scalar.copy(out=_sap(vs, 64, [[65, 8]]), in_=ev[:])
                # scores + softmax + P.V per 4-head group
                O = work.tile([128, 512], bf, tag="ob")
                R = work.tile([128, 8], f32, tag="rc")
                for g in range(2):
                    psSt = ps1.tile([128, 512], f32, tag="st")
                    for hi in range(4):
                        h = g * 4 + hi
                        hc, hh = h // 2, (h % 2) * 64
                        nc.tensor.matmul(
                            psSt[:, hi * 128:(hi + 1) * 128],
                            kt[hh:hh + 64, hc, :], qt[hh:hh + 64, hc, :],
                            start=True, stop=False)
                        nc.tensor.matmul(
                            psSt[:, hi * 128:(hi + 1) * 128],
                            mbt[:], mbt[:], start=False, stop=True)
                    em = work.tile([128, 4, 128], bf, tag="em")
                    nc.scalar.activation(_sap(em, 0, [[1, 512]]), psSt[:],
                                         Exp, scale=SCALE)
                    psPV = ps1.tile([128, 512], f32, tag="pv")
                    for hi in range(4):
                        h = g * 4 + hi
                        nc.tensor.matmul(
                            psPV[:, hi * 65:hi * 65 + 65],
                            em[:, hi, :], vs[:, h, :], start=True, stop=True)
                    nc.vector.reciprocal(
                        R[:, g * 4:(g + 1) * 4], _sap(psPV, 64, [[65, 4]]))
                    nc.vector.tensor_mul(
                        _sap(O, g * 256, [[64, 4], [1, 64]]),
                        _sap(psPV, 0, [[65, 4], [1, 64]]),
                        _sap(R, g * 4, [[1, 4], [0, 64]]))
                return O

            # ---- local path: 16 tiles ----
            for I in range(16):
                praw = work.tile([128, 2048], bf, tag="pl")
                nc.sync.dma_start(out=praw, in_=_dap(
                    xs, I * 262144, [[2048, 128], [1, 2048]]))
                # pool: sum taps (ta: +128?) praw free = (k,e,ta,fb128)
                p1 = work.tile([128, 1024], bf, tag="p1")
                nc.vector.tensor_add(
                    _sap(p1, 0, [[256, 4], [128, 2], [1, 128]]),
                    _sap(praw, 0, [[512, 4], [256, 2], [1, 128]]),
                    _sap(praw, 128, [[512, 4], [256, 2], [1, 128]]))
                xp = work.tile([128, 512], bf, tag="xg")
                nc.vector.tensor_add(
                    _sap(xp, 0, [[128, 4], [64, 2], [1, 64]]),
                    _sap(p1, 0, [[256, 4], [128, 2], [2, 64]]),
                    _sap(p1, 1, [[256, 4], [128, 2], [2, 64]]))
                O = emit_attn(xp, wl, add_bv=True)
                nc.sync.dma_start(
                    out=_dap(lout, I * 65536,
                             [[32768, 2], [1024, 32], [512, 2], [1, 512]]),
                    in_=O)

            # ---- smoothing + bilinear x2 upsample (channel-major) ----
            # a1[y] = l[y-1]+l[y] (reflect y=0), a2[x] = l[x]+l[x+1]
            # (reflect x=63), sraw = a1+a2; u'[2y] = sraw[y]+sraw[y-1]/3,
            # u'[2y+1] = sraw[y]+sraw[y+1]/3 (clamped); same along x; then
            # lup = 0.28125*L' + obias (global-path out bias folded here).
            for cc in range(2):
                for st in range(4):          # strips of 16 pooled rows
                    y0 = st * 16
                    r0, r1 = max(y0 - 2, 0), min(y0 + 17, 64)   # Lp rows
                    s0, s1 = max(y0 - 1, 0), min(y0 + 17, 64)   # sraw rows
                    nlr = r1 - r0
                    nsr = s1 - s0
                    Lp = work.tile([128, nlr * 64], bf, tag="lp")
                    nc.sync.dma_start(out=Lp, in_=_dap(
                        lout, cc * 128 * 4096 + r0 * 64,
                        [[4096, 128], [1, nlr * 64]]))

                    def lrow(y):
                        return (y - r0) * 64

                    def srow(y):
                        return (y - s0) * 64

                    a1 = work.tile([128, nsr * 64], bf, tag="a1")
                    ym = max(s0, 1)  # main region rows [ym, s1)
                    nc.vector.tensor_add(
                        _sap(a1, srow(ym), [[1, (s1 - ym) * 64]]),
                        _sap(Lp, lrow(ym - 1), [[1, (s1 - ym) * 64]]),
                        _sap(Lp, lrow(ym), [[1, (s1 - ym) * 64]]))
                    if s0 == 0:  # reflect top: a1[0] = l[0] + l[1]
                        nc.vector.tensor_add(
                            _sap(a1, 0, [[1, 64]]),
                            _sap(Lp, 0, [[1, 64]]),
                            _sap(Lp, 64, [[1, 64]]))
                    a2 = work.tile([128, nsr * 64], bf, tag="a2")
                    nc.gpsimd.tensor_add(
                        _sap(a2, 0, [[64, nsr], [1, 63]]),
                        _sap(Lp, lrow(s0), [[64, nsr], [1, 63]]),
                        _sap(Lp, lrow(s0) + 1, [[64, nsr], [1, 63]]))
                    nc.gpsimd.tensor_add(
                        _sap(a2, 63, [[64, nsr]]),
                        _sap(Lp, lrow(s0) + 63, [[64, nsr]]),
                        _sap(Lp, lrow(s0) + 62, [[64, nsr]]))
                    sraw = work.tile([128, nsr * 64], bf, tag="sr")
                    nc.vector.tensor_add(sraw[:], a1[:], a2[:])
                    # y-upsample (u' rows Y-2*y0, 32 rows x 64 cols)
                    u = work.tile([128, 2048], bf, tag="uu")
                    ye = max(y0, 1)  # even rows needing y-1
                    nc.vector.scalar_tensor_tensor(
                        out=_sap(u, (ye - y0) * 128,
                                 [[128, y0 + 16 - ye], [1, 64]]),
                        in0=_sap(sraw, srow(ye - 1),
                                 [[64, y0 + 16 - ye], [1, 64]]),
                        scalar=THIRD,
                        in1=_sap(sraw, srow(ye), [[64, y0 + 16 - ye], [1, 64]]),
                        op0=MUL, op1=ADD)
                    if y0 == 0:  # Y=0: taps both row 0
                        nc.vector.scalar_tensor_tensor(
                            out=_sap(u, 0, [[1, 64]]),
                            in0=_sap(sraw, 0, [[1, 64]]), scalar=THIRD,
                            in1=_sap(sraw, 0, [[1, 64]]), op0=MUL, op1=ADD)
                    yo1 = min(y0 + 16, 63)  # odd rows needing y+1
                    nc.vector.scalar_tensor_tensor(
                        out=_sap(u, 64, [[128, yo1 - y0], [1, 64]]),
                        in0=_sap(sraw, srow(y0 + 1), [[64, yo1 - y0], [1, 64]]),
                        scalar=THIRD,
                        in1=_sap(sraw, srow(y0), [[64, yo1 - y0], [1, 64]]),
                        op0=MUL, op1=ADD)
                    if y0 + 16 == 64:  # Y=127: taps both row 63
                        nc.vector.scalar_tensor_tensor(
                            out=_sap(u, 31 * 64, [[1, 64]]),
                            in0=_sap(sraw, srow(63), [[1, 64]]), scalar=THIRD,
                            in1=_sap(sraw, srow(63), [[1, 64]]),
                            op0=MUL, op1=ADD)
                    # x-upsample per 16-row half + 0.28125 scale + obias
                    for hf in range(2):
                        Lh = work.tile([128, 2048], bf, tag="lh")
                        ub = hf * 16 * 64
                        nc.vector.scalar_tensor_tensor(
                            out=_sap(Lh, 2, [[128, 16], [2, 63]]),
                            in0=_sap(u, ub, [[64, 16], [1, 63]]), scalar=THIRD,
                            in1=_sap(u, ub + 1, [[64, 16], [1, 63]]),
                            op0=MUL, op1=ADD)
                        nc.vector.scalar_tensor_tensor(
                            out=_sap(Lh, 0, [[128, 16]]),
                            in0=_sap(u, ub, [[64, 16]]), scalar=THIRD,
                            in1=_sap(u, ub, [[64, 16]]), op0=MUL, op1=ADD)
                        nc.vector.scalar_tensor_tensor(
                            out=_sap(Lh, 1, [[128, 16], [2, 63]]),
                            in0=_sap(u, ub + 1, [[64, 16], [1, 63]]),
                            scalar=THIRD,
                            in1=_sap(u, ub, [[64, 16], [1, 63]]),
                            op0=MUL, op1=ADD)
                        nc.vector.scalar_tensor_tensor(
                            out=_sap(Lh, 127, [[128, 16]]),
                            in0=_sap(u, ub + 63, [[64, 16]]), scalar=THIRD,
                            in1=_sap(u, ub + 63, [[64, 16]]), op0=MUL, op1=ADD)
                        Lsc = work.tile([128, 2048], bf, tag="ls")
                        nc.vector.scalar_tensor_tensor(
                            out=_sap(Lsc, 0, [[512, 4], [128, 4], [1, 128]]),
                            in0=_sap(Lh, 0, [[512, 4], [128, 4], [1, 128]]),
                            scalar=0.28125,
                            in1=_sap(obb, 0, [[0, 4], [128, 4], [1, 128]]),
                            op0=MUL, op1=ADD)
                        nc.sync.dma_start(
                            out=_dap(lup,
                                     cc * 128 * 16384
                                     + (2 * y0 + 16 * hf) * 128,
                                     [[16384, 128], [1, 2048]]),
                            in_=Lsc)

            # ---- global path: 64 tiles (I in [0,32), Jh in {0,1}) ----
            for ti in range(64):
                I, Jh = ti // 2, ti % 2
                toff = (I * 256 + Jh * 64) * 512
                x_t = work.tile([128, 512], bf, tag="xg")
                nc.sync.dma_start(out=x_t, in_=_dap(
                    xs, toff,
                    [[65536, 2], [1024, 32], [512, 2], [1, 512]]))
                O = emit_attn(x_t, wg, add_bv=False)
                # proj
                psOt = ps2.tile([128, 512], bf, tag="tr")
                for kc in range(4):
                    nc.tensor.transpose(
                        psOt[:, kc * 128:(kc + 1) * 128],
                        O[:, kc * 128:(kc + 1) * 128], identb[:])
                ot = work.tile([128, 4, 128], bf, tag="ot")
                nc.scalar.copy(out=_sap(ot, 0, [[1, 512]]), in_=psOt[:])
                psP = ps2.tile([128, 512], f32, tag="mm")
                for kc in range(4):
                    nc.tensor.matmul(psP[:], ot[:, kc, :], wpj[:, kc, :],
                                     start=(kc == 0), stop=(kc == 3))
                lupt = work.tile([128, 512], bf, tag="lu")
                nc.sync.dma_start(out=lupt, in_=_dap(
                    lup, toff,
                    [[65536, 2], [1024, 32], [512, 2], [1, 512]]))
                t1 = work.tile([128, 512], f32, tag="t1")
                nc.vector.tensor_add(t1[:], psP[:], lupt[:])
                psF = ps1.tile([128, 512], f32, tag="trf")
                for kc in range(4):
                    nc.tensor.transpose(
                        psF[:, kc * 128:(kc + 1) * 128],
                        t1[:, kc * 128:(kc + 1) * 128], identf[:])
                fin = work.tile([128, 4, 128], f32, tag="fin")
                nc.scalar.copy(out=_sap(fin, 0, [[1, 512]]), in_=psF[:])
                for di in range(2):
                    nc.sync.dma_start(
                        out=_dap(out, I * 256 + di * 128 + Jh * 64,
                                 [[8192, 128], [1048576, 4], [1, 64]]),
                        in_=_sap(fin, di * 64, [[128, 4], [1, 64]]))
    return nc


def _get_nc():
    if 'nc' not in _RUN_CACHE:
        _RUN_CACHE['nc'] = _build_nc()
    return _RUN_CACHE['nc']


def make_in_maps(inputs):
    import ml_dtypes
    bf16 = ml_dtypes.bfloat16
    x = np.asarray(inputs['x'], dtype=np.float32)
    Wqkv = np.asarray(inputs['Wqkv'], dtype=np.float32)
    bqkv = np.asarray(inputs['bqkv'], dtype=np.float32)
    Wproj = np.asarray(inputs['Wproj'], dtype=np.float32)
    bproj = np.asarray(inputs['bproj'], dtype=np.float32)

    bq, bv = bqkv[:512], bqkv[1024:]
    wqkv_g = np.ascontiguousarray(Wqkv.astype(bf16)).reshape(-1)
    wqkv_l = np.ascontiguousarray((0.25 * Wqkv).astype(bf16)).reshape(-1)
    wproj = np.ascontiguousarray(Wproj.astype(bf16)).reshape(-1)
    bqmat = np.zeros((512, 8), np.float32)
    bqmat[np.arange(512), np.arange(512) // 64] = bq
    bqmat = bqmat.astype(bf16).reshape(-1)
    p = np.arange(128)
    maskb = np.zeros((32, 128), np.float32)
    maskb[(p % 64) // 2, p] = MSQ
    maskb = maskb.astype(bf16).reshape(-1)
    obias = (bproj + bv @ Wproj).astype(bf16)
    bvv = bv.astype(bf16)

    in_maps = []
    shards = []
    for b in range(B):
        for half in range(2):
            shards.append((b, half))
            xs = np.ascontiguousarray(
                x[b, 256 * half:256 * (half + 1)]).reshape(-1).astype(bf16)
            in_maps.append({
                "xs": xs, "wqkv_g": wqkv_g, "wqkv_l": wqkv_l,
                "wproj": wproj, "bqmat": bqmat, "maskb": maskb,
                "obias": obias, "bvv": bvv,
            })
    return in_maps, shards


def kernel(**inputs):
    from concourse.bass_utils import run_bass_kernel_spmd
    nc = _get_nc()
    in_maps, shards = make_in_maps(inputs)
    r = run_bass_kernel_spmd(nc, in_maps, core_ids=list(range(8)))
    _RUN_CACHE['last_result'] = r
    full = np.empty((B, C, H, W), dtype=np.float32)
    for (b, half), res in zip(shards, r.results):
        full[b, :, 64 * half:64 * (half + 1), :] = res["out"]
    return full


# revision 24
# speedup vs baseline: 1.1620x; 1.0692x over previous
"""Bi-path windowed attention kernel for Trainium2 (8 NeuronCores), v2.

Problem: x (4, 512, 128, 128) f32. Reference (per batch): raw-reshape to
tokens (128,128,512); global path = 2x2-window MHA (8 heads, hd=64) +
out-proj; local path = AvgPool2(x) -> raw-reshape tokens (64,64,512) ->
2x2-window MHA -> raw-reshape -> reflect-pad smoothing -> bilinear x2
upsample; out = (global + local) transposed to (B, C, H, W).

Sharding: 8 shards = batch (4) x channel-half (2); channel half h of x ==
token rows [64h, 64h+64), and both paths for those rows stay inside the
shard, so shards are independent.

v2 design (vs v1 elementwise attention): tokens-on-partitions, all-bf16
matmul pipeline with the attention itself on the PE:
 - tile = 32 windows x 4 tokens = 128 partitions (p = di*64 + 2w + dj)
 - Q^T/K^T computed directly in [head-dim, token] layout (lhsT = Wq/Wk
   chunks, rhs = x^T chunks); V token-major.
 - scores: per head one [64]-contraction matmul giving all 128x128 token
   pairs of the 32-window group, plus a rank-32 accumulation matmul that
   adds +C to same-window pairs (uniform boost cancels in softmax; the
   off-window pairs stay ~exp(-C/8) smaller = masked).
 - softmax without max-subtraction; q-bias cancels in softmax, k-bias
   becomes a per-token factor ev = exp(scale*K~_h.bq_h) folded into V and
   into a 65th "ones" column that makes the P.V matmul also emit the
   softmax denominator. v-bias: global path folds bv@Wproj+bproj into an
   output bias added via the smoothing stage; local path adds bv to V.
 - local pooling is 4 contiguous-token adds in token-major layout (the
   raw reshape makes pooled-token gathers contiguous in DRAM).
 - layout exchanges (token-major <-> channel-major) are free flat
   reinterpretations of DRAM scratch.
"""
import sys
if '/opt/trn_rl_repo' not in sys.path:
    sys.path.append('/opt/trn_rl_repo')
import numpy as np

_RUN_CACHE = {}

B, C, H, W = 4, 512, 128, 128
NH, HD = 8, 64
SCALE = float(HD) ** -0.5
MSQ = 11.3125   # bf16(sqrt(128)); MSQ^2 ~ 128 uniform in-window boost


def _mk_tile_context_fixed():
    import concourse.mybir as mybir
    import concourse.tile as tile
    from concourse.vector_clock import ScopedClock, VectorClock

    class TileContextFixed(tile.TileContext):
        """Works around a walrus codegen limit in this toolchain: max ONE
        sync-wait per instruction. Extra waits are peeled onto single-wait
        NoOps on the same engine; the kernel-tail drain gets per-proc
        single-wait NOPs instead of one multi-wait drain."""
        _ctr = 0

        def _lower_ordered_insts(self, ordered):
            cls = type(self)
            for bb_name, insts in ordered.items():
                new_list = []
                for inst in insts:
                    try:
                        si = inst.sync_info
                    except Exception:
                        si = None
                    if si is not None and len(si.on_wait) > 1:
                        waits = list(si.on_wait)
                        extra, keep = waits[:-1], waits[-1:]
                        for w in extra:
                            nop = mybir.InstNoOp(
                                name=f"I-waitsplit-{cls._ctr}", ins=[], outs=[])
                            cls._ctr += 1
                            nop.engine = inst.engine
                            nop.sync_info = mybir.SyncInfo(
                                on_wait=[w], on_update=[])
                            self.nc.register_instruction(nop, overwrite=True)
                            new_list.append(nop)
                        inst.sync_info = mybir.SyncInfo(
                            on_wait=keep, on_update=list(si.on_update))
                    new_list.append(inst)
                ordered[bb_name] = new_list
            super()._lower_ordered_insts(ordered)

        def _drain_and_barrier(self, tick_clock, wait_clock):
            gc = tick_clock.global_clock
            scoped = gc if hasattr(gc, 'items') else ScopedClock({None: gc})
            for scope, vc in scoped.items():
                n = len(vc)
                for proc in range(n):
                    t = vc[proc]
                    if t <= 0:
                        continue
                    vec = [0] * n
                    vec[proc] = t
                    nop = self.nc.sync.nop()
                    wait_clock.add_sem_waits(
                        nop.ins, ScopedClock({scope: VectorClock(vec)}))
            self.nc.sync.drain()
            self.nc.all_engine_barrier()
            popped = self.nc._tile_sem_poison_stack.pop()
            assert popped is self._sem_poison
            self.nc.clear_and_free_semaphores(
                list(self.sems.allocated().values()))
            self.nc.all_engine_barrier()

    return TileContextFixed


def _dap(handle, off, dims):
    """Raw DRAM access pattern: flat element offset + [step, count] dims."""
    import concourse.bass as bass
    base = handle[:]
    return bass.AP(tensor=base.tensor, offset=base.offset + off,
                   ap=[list(d) for d in dims])


def _sap(tile_, off, dims):
    """SBUF tile sub-AP: keep partition dim, replace free dims."""
    import concourse.bass as bass
    base = tile_[:]
    return bass.AP(tensor=base.tensor, offset=base.offset + off,
                   ap=[list(base.ap[0])] + [list(d) for d in dims])


def _build_nc():
    import concourse.bass as bass
    import concourse.mybir as mybir
    from concourse.masks import make_identity
    TileContextFixed = _mk_tile_context_fixed()
    f32 = mybir.dt.float32
    bf = mybir.dt.bfloat16
    Copy = mybir.ActivationFunctionType.Copy
    Exp = mybir.ActivationFunctionType.Exp
    ADD = mybir.AluOpType.add
    MUL = mybir.AluOpType.mult
    THIRD = 1.0 / 3.0

    nc = bass.Bass()
    xs = nc.declare_dram_parameter("xs", [8192 * 512], bf, isOutput=False)
    wg_d = nc.declare_dram_parameter("wqkv_g", [C * 3 * C], bf, isOutput=False)
    wl_d = nc.declare_dram_parameter("wqkv_l", [C * 3 * C], bf, isOutput=False)
    wp_d = nc.declare_dram_parameter("wproj", [C * C], bf, isOutput=False)
    bq_d = nc.declare_dram_parameter("bqmat", [C * 8], bf, isOutput=False)
    mb_d = nc.declare_dram_parameter("maskb", [32 * 128], bf, isOutput=False)
    ob_d = nc.declare_dram_parameter("obias", [C], bf, isOutput=False)
    bv_d = nc.declare_dram_parameter("bvv", [C], bf, isOutput=False)
    out = nc.declare_dram_parameter("out", [C, 64, W], f32, isOutput=True)

    lout = nc.dram_tensor("lout", [2048 * 512], bf)
    lup = nc.dram_tensor("lup", [8192 * 512], bf)

    with TileContextFixed(nc) as tc:
        with (
            tc.tile_pool(name="consts", bufs=1) as consts,
            tc.tile_pool(name="work", bufs=2) as work,
            tc.tile_pool(name="work3", bufs=3) as work3,
            tc.tile_pool(name="ps2", bufs=2, space="PSUM") as ps2,
            tc.tile_pool(name="ps1", bufs=1, space="PSUM") as ps1,
        ):
            # ---- constants ----
            identb = consts.tile([128, 128], bf)
            make_identity(nc, identb[:])
            identf = consts.tile([128, 128], f32)
            make_identity(nc, identf[:])
            wg = consts.tile([128, 4, 1536], bf)
            nc.sync.dma_start(out=wg, in_=_dap(
                wg_d, 0, [[1536, 128], [196608, 4], [1, 1536]]))
            wl = consts.tile([128, 4, 1536], bf)
            nc.sync.dma_start(out=wl, in_=_dap(
                wl_d, 0, [[1536, 128], [196608, 4], [1, 1536]]))
            wpj = consts.tile([128, 4, 512], bf)
            nc.sync.dma_start(out=wpj, in_=_dap(
                wp_d, 0, [[512, 128], [65536, 4], [1, 512]]))
            bqb = consts.tile([128, 4, 8], bf)
            nc.sync.dma_start(out=bqb, in_=_dap(
                bq_d, 0, [[8, 128], [1024, 4], [1, 8]]))
            mbt = consts.tile([32, 128], bf)
            nc.sync.dma_start(out=mbt, in_=_dap(mb_d, 0, [[128, 32], [1, 128]]))
            obb = consts.tile([128, 512], bf)
            nc.sync.dma_start(out=obb, in_=_dap(ob_d, 0, [[0, 128], [1, 512]]))
            bvb = consts.tile([128, 512], bf)
            nc.sync.dma_start(out=bvb, in_=_dap(bv_d, 0, [[0, 128], [1, 512]]))

            # ---- stage emitters (software-pipelined across tiles) ----
            # xt: [128 c-in-chunk, 4 chunk, 128 tok] bf16 = x^T for a tile
            # of 128 tokens (p = di*64 + 2w + dj).
            def stage_qkv(xt, wt, add_bv):
                # QT/KT: [hd, tok] per 128-chunk of (h,d)
                psQT = ps2.tile([128, 512], f32, tag="mm")
                for hc in range(4):
                    for kc in range(4):
                        nc.tensor.matmul(
                            psQT[:, hc * 128:(hc + 1) * 128],
                            wt[:, kc, hc * 128:(hc + 1) * 128],
                            xt[:, kc, :], start=(kc == 0), stop=(kc == 3))
                qt = work3.tile([128, 4, 128], bf, tag="qt")
                nc.scalar.copy(out=_sap(qt, 0, [[1, 512]]), in_=psQT[:])
                psKT = ps2.tile([128, 512], f32, tag="mm")
                for hc in range(4):
                    for kc in range(4):
                        nc.tensor.matmul(
                            psKT[:, hc * 128:(hc + 1) * 128],
                            wt[:, kc, 512 + hc * 128:512 + (hc + 1) * 128],
                            xt[:, kc, :], start=(kc == 0), stop=(kc == 3))
                kt = work3.tile([128, 4, 128], bf, tag="kt")
                nc.scalar.copy(out=_sap(kt, 0, [[1, 512]]), in_=psKT[:])
                # kb[tok, h] = K~_h . bq_h  (psum bank shared with finT)
                psKB = ps1.tile([128, 512], f32, tag="kbtrf")
                for hc in range(4):
                    nc.tensor.matmul(psKB[:, 0:8], kt[:, hc, :],
                                     bqb[:, hc, :],
                                     start=(hc == 0), stop=(hc == 3))
                ev = work.tile([128, 8], f32, tag="ev")
                nc.scalar.activation(ev[:], psKB[:, 0:8], Exp, scale=SCALE)
                # V (after QT/KT so the "mm" rotation can't deadlock)
                psV = ps2.tile([128, 512], f32, tag="mm")
                for kc in range(4):
                    nc.tensor.matmul(psV[:], xt[:, kc, :],
                                     wt[:, kc, 1024:1536],
                                     start=(kc == 0), stop=(kc == 3))
                vs = work3.tile([128, 8, 65], bf, tag="vs")
                if add_bv:
                    vb = work.tile([128, 512], f32, tag="vb")
                    nc.vector.tensor_add(vb[:], psV[:], bvb[:])
                    nc.vector.tensor_mul(
                        _sap(vs, 0, [[65, 8], [1, 64]]),
                        _sap(vb, 0, [[64, 8], [1, 64]]),
                        _sap(ev, 0, [[1, 8], [0, 64]]))
                else:
                    nc.vector.tensor_mul(
                        _sap(vs, 0, [[65, 8], [1, 64]]),
                        _sap(psV, 0, [[64, 8], [1, 64]]),
                        _sap(ev, 0, [[1, 8], [0, 64]]))
                nc.scalar.copy(out=_sap(vs, 64, [[65, 8]]), in_=ev[:])
                return qt, kt, vs

            def stage_scores(qkv):
                qt, kt, vs = qkv
                # scores for both 4-head groups (mask accs grouped so their
                # stationary reloads back-to-back on HW)
                psSts = []
                for g in range(2):
                    psSt = ps2.tile([128, 512], f32, tag="st")
                    for hi in range(4):
                        h = g * 4 + hi
                        hc, hh = h // 2, (h % 2) * 64
                        nc.tensor.matmul(
                            psSt[:, hi * 128:(hi + 1) * 128],
                            kt[hh:hh + 64, hc, :], qt[hh:hh + 64, hc, :],
                            start=True, stop=False)
                        nc.tensor.matmul(
                            psSt[:, hi * 128:(hi + 1) * 128],
                            mbt[:], mbt[:], start=False, stop=True)
                    psSts.append(psSt)
                return psSts

            def stage_pv(qkv, psSts):
                qt, kt, vs = qkv
                ems = []
                for g in range(2):
                    em = work.tile([128, 4, 128], bf, tag="em")
                    nc.scalar.activation(_sap(em, 0, [[1, 512]]), psSts[g][:],
                                         Exp, scale=SCALE)
                    ems.append(em)
                O = work.tile([128, 512], bf, tag="ob")
                R = work.tile([128, 8], f32, tag="rc")
                psPVs = []
                for g in range(2):
                    psPV = ps2.tile([128, 512], f32, tag="pv")
                    for hi in range(4):
                        h = g * 4 + hi
                        nc.tensor.matmul(
                            psPV[:, hi * 65:hi * 65 + 65],
                            ems[g][:, hi, :], vs[:, h, :],
                            start=True, stop=True)
                    psPVs.append(psPV)
                for g in range(2):
                    nc.vector.reciprocal(
                        R[:, g * 4:(g + 1) * 4], _sap(psPVs[g], 64, [[65, 4]]))
                    nc.vector.tensor_mul(
                        _sap(O, g * 256, [[64, 4], [1, 64]]),
                        _sap(psPVs[g], 0, [[65, 4], [1, 64]]),
                        _sap(R, g * 4, [[1, 4], [0, 64]]))
                return O

            def stage_soft(qkv):
                return stage_pv(qkv, stage_scores(qkv))

            # ---- local path: 16 tiles, 1-tile software pipeline ----
            def local_load(I):
                praw = work.tile([128, 2048], bf, tag="pl")
                nc.sync.dma_start(out=praw, in_=_dap(
                    xs, I * 262144, [[2048, 128], [1, 2048]]))
                # pool taps: praw free = (k, e, ta, f*tb)
                p1 = work.tile([128, 1024], bf, tag="p1")
                nc.vector.tensor_add(
                    _sap(p1, 0, [[256, 4], [128, 2], [1, 128]]),
                    _sap(praw, 0, [[512, 4], [256, 2], [1, 128]]),
                    _sap(praw, 128, [[512, 4], [256, 2], [1, 128]]))
                xp = work.tile([128, 512], bf, tag="xg")
                nc.vector.tensor_add(
                    _sap(xp, 0, [[128, 4], [64, 2], [1, 64]]),
                    _sap(p1, 0, [[256, 4], [128, 2], [2, 64]]),
                    _sap(p1, 1, [[256, 4], [128, 2], [2, 64]]))
                psT = ps1.tile([128, 512], bf, tag="tr")
                for kc in range(4):
                    nc.tensor.transpose(
                        psT[:, kc * 128:(kc + 1) * 128],
                        xp[:, kc * 128:(kc + 1) * 128], identb[:])
                xt = work3.tile([128, 4, 128], bf, tag="xt")
                nc.scalar.copy(out=_sap(xt, 0, [[1, 512]]), in_=psT[:])
                return xt

            def local_out(I, O):
                nc.sync.dma_start(
                    out=_dap(lout, I * 65536,
                             [[32768, 2], [1024, 32], [512, 2], [1, 512]]),
                    in_=O)

            # ---- smoothing + bilinear x2 upsample (channel-major) ----
            # a1[y] = l[y-1]+l[y] (reflect y=0), a2[x] = l[x]+l[x+1]
            # (reflect x=63), sraw = a1+a2; u'[2y] = sraw[y]+sraw[y-1]/3,
            # u'[2y+1] = sraw[y]+sraw[y+1]/3 (clamped); same along x; then
            # lup = 0.28125*L' + obias (global-path out bias folded here).
            # Emitted as strips interleaved into the local/global pipelines.
            def smooth_strip(cc, st):
                    y0 = st * 16
                    r0, r1 = max(y0 - 2, 0), min(y0 + 17, 64)   # Lp rows
                    s0, s1 = max(y0 - 1, 0), min(y0 + 17, 64)   # sraw rows
                    nlr = r1 - r0
                    nsr = s1 - s0
                    Lp = work.tile([128, nlr * 64], bf, tag="lp")
                    nc.sync.dma_start(out=Lp, in_=_dap(
                        lout, cc * 128 * 4096 + r0 * 64,
                        [[4096, 128], [1, nlr * 64]]))

                    def lrow(y):
                        return (y - r0) * 64

                    def srow(y):
                        return (y - s0) * 64

                    a1 = work.tile([128, nsr * 64], bf, tag="a1")
                    ym = max(s0, 1)  # main region rows [ym, s1)
                    nc.vector.tensor_add(
                        _sap(a1, srow(ym), [[1, (s1 - ym) * 64]]),
                        _sap(Lp, lrow(ym - 1), [[1, (s1 - ym) * 64]]),
                        _sap(Lp, lrow(ym), [[1, (s1 - ym) * 64]]))
                    if s0 == 0:  # reflect top: a1[0] = l[0] + l[1]
                        nc.vector.tensor_add(
                            _sap(a1, 0, [[1, 64]]),
                            _sap(Lp, 0, [[1, 64]]),
                            _sap(Lp, 64, [[1, 64]]))
                    a2 = work.tile([128, nsr * 64], bf, tag="a2")
                    nc.gpsimd.tensor_add(
                        _sap(a2, 0, [[64, nsr], [1, 63]]),
                        _sap(Lp, lrow(s0), [[64, nsr], [1, 63]]),
                        _sap(Lp, lrow(s0) + 1, [[64, nsr], [1, 63]]))
                    nc.gpsimd.tensor_add(
                        _sap(a2, 63, [[64, nsr]]),
                        _sap(Lp, lrow(s0) + 63, [[64, nsr]]),
                        _sap(Lp, lrow(s0) + 62, [[64, nsr]]))
                    sraw = work.tile([128, nsr * 64], bf, tag="sr")
                    nc.vector.tensor_add(sraw[:], a1[:], a2[:])
                    # y-upsample (u' rows Y-2*y0, 32 rows x 64 cols)
                    u = work.tile([128, 2048], bf, tag="uu")
                    ye = max(y0, 1)  # even rows needing y-1
                    nc.vector.scalar_tensor_tensor(
                        out=_sap(u, (ye - y0) * 128,
                                 [[128, y0 + 16 - ye], [1, 64]]),
                        in0=_sap(sraw, srow(ye - 1),
                                 [[64, y0 + 16 - ye], [1, 64]]),
                        scalar=THIRD,
                        in1=_sap(sraw, srow(ye), [[64, y0 + 16 - ye], [1, 64]]),
                        op0=MUL, op1=ADD)
                    if y0 == 0:  # Y=0: taps both row 0
                        nc.vector.scalar_tensor_tensor(
                            out=_sap(u, 0, [[1, 64]]),
                            in0=_sap(sraw, 0, [[1, 64]]), scalar=THIRD,
                            in1=_sap(sraw, 0, [[1, 64]]), op0=MUL, op1=ADD)
                    yo1 = min(y0 + 16, 63)  # odd rows needing y+1
                    nc.vector.scalar_tensor_tensor(
                        out=_sap(u, 64, [[128, yo1 - y0], [1, 64]]),
                        in0=_sap(sraw, srow(y0 + 1), [[64, yo1 - y0], [1, 64]]),
                        scalar=THIRD,
                        in1=_sap(sraw, srow(y0), [[64, yo1 - y0], [1, 64]]),
                        op0=MUL, op1=ADD)
                    if y0 + 16 == 64:  # Y=127: taps both row 63
                        nc.vector.scalar_tensor_tensor(
                            out=_sap(u, 31 * 64, [[1, 64]]),
                            in0=_sap(sraw, srow(63), [[1, 64]]), scalar=THIRD,
                            in1=_sap(sraw, srow(63), [[1, 64]]),
                            op0=MUL, op1=ADD)
                    # x-upsample per 16-row half + 0.28125 scale + obias
                    for hf in range(2):
                        Lh = work.tile([128, 2048], bf, tag="lh")
                        ub = hf * 16 * 64
                        nc.vector.scalar_tensor_tensor(
                            out=_sap(Lh, 2, [[128, 16], [2, 63]]),
                            in0=_sap(u, ub, [[64, 16], [1, 63]]), scalar=THIRD,
                            in1=_sap(u, ub + 1, [[64, 16], [1, 63]]),
                            op0=MUL, op1=ADD)
                        nc.vector.scalar_tensor_tensor(
                            out=_sap(Lh, 0, [[128, 16]]),
                            in0=_sap(u, ub, [[64, 16]]), scalar=THIRD,
                            in1=_sap(u, ub, [[64, 16]]), op0=MUL, op1=ADD)
                        nc.vector.scalar_tensor_tensor(
                            out=_sap(Lh, 1, [[128, 16], [2, 63]]),
                            in0=_sap(u, ub + 1, [[64, 16], [1, 63]]),
                            scalar=THIRD,
                            in1=_sap(u, ub, [[64, 16], [1, 63]]),
                            op0=MUL, op1=ADD)
                        nc.vector.scalar_tensor_tensor(
                            out=_sap(Lh, 127, [[128, 16]]),
                            in0=_sap(u, ub + 63, [[64, 16]]), scalar=THIRD,
                            in1=_sap(u, ub + 63, [[64, 16]]), op0=MUL, op1=ADD)
                        Lsc = work.tile([128, 2048], bf, tag="ls")
                        nc.vector.scalar_tensor_tensor(
                            out=_sap(Lsc, 0, [[512, 4], [128, 4], [1, 128]]),
                            in0=_sap(Lh, 0, [[512, 4], [128, 4], [1, 128]]),
                            scalar=0.28125,
                            in1=_sap(obb, 0, [[0, 4], [128, 4], [1, 128]]),
                            op0=MUL, op1=ADD)
                        nc.sync.dma_start(
                            out=_dap(lup,
                                     cc * 128 * 16384
                                     + (2 * y0 + 16 * hf) * 128,
                                     [[16384, 128], [1, 2048]]),
                            in_=Lsc)

            # ---- local path: 16 tiles, 2-lag pipeline; cc0 smoothing
            # strips interleave once their lout chunk (tiles 0-7) is done.
            lx = {0: local_load(0), 1: local_load(1)}
            lq = {0: stage_qkv(lx.pop(0), wl, True)}
            for I in range(16):
                if I + 2 < 16:
                    lx[I + 2] = local_load(I + 2)
                if I + 1 < 16:
                    lq[I + 1] = stage_qkv(lx.pop(I + 1), wl, True)
                local_out(I, stage_soft(lq.pop(I)))
                if I in (9, 11, 13, 15):
                    smooth_strip(0, (I - 9) // 2)

            # ---- global path: 64 tiles (I in [0,32), Jh in {0,1}),
            # software-pipelined: x-prefetch lag 2, qkv lag 1; cc1
            # smoothing strips interleave into the first tiles ----
            def gx_load(ti):
                I, Jh = ti // 2, ti % 2
                toff = (I * 256 + Jh * 64) * 512
                xt = work3.tile([128, 4, 128], bf, tag="xt")
                for di in range(2):
                    nc.sync.dma_start_transpose(
                        out=xt[:, :, di * 64:(di + 1) * 64],
                        in_=_dap(xs, toff + di * 65536,
                                 [[512, 64], [1, 512]]))
                return xt

            def lup_load(ti):
                I, Jh = ti // 2, ti % 2
                toff = (I * 256 + Jh * 64) * 512
                lupt = work.tile([128, 512], bf, tag="lu")
                nc.sync.dma_start(out=lupt, in_=_dap(
                    lup, toff,
                    [[65536, 2], [1024, 32], [512, 2], [1, 512]]))
                return lupt

            def g_proj(O, lupt):
                psOt = ps1.tile([128, 512], bf, tag="tr")
                for kc in range(4):
                    nc.tensor.transpose(
                        psOt[:, kc * 128:(kc + 1) * 128],
                        O[:, kc * 128:(kc + 1) * 128], identb[:])
                ot = work.tile([128, 4, 128], bf, tag="ot")
                nc.scalar.copy(out=_sap(ot, 0, [[1, 512]]), in_=psOt[:])
                psP = ps2.tile([128, 512], f32, tag="mm")
                for kc in range(4):
                    nc.tensor.matmul(psP[:], ot[:, kc, :], wpj[:, kc, :],
                                     start=(kc == 0), stop=(kc == 3))
                t1 = work.tile([128, 512], f32, tag="t1")
                nc.vector.tensor_add(t1[:], psP[:], lupt[:])
                return t1

            def g_fin(ti, t1):
                I, Jh = ti // 2, ti % 2
                psF = ps1.tile([128, 512], f32, tag="kbtrf")
                for kc in range(4):
                    nc.tensor.transpose(
                        psF[:, kc * 128:(kc + 1) * 128],
                        t1[:, kc * 128:(kc + 1) * 128], identf[:])
                fin = work.tile([128, 4, 128], f32, tag="fin")
                nc.scalar.copy(out=_sap(fin, 0, [[1, 512]]), in_=psF[:])
                for di in range(2):
                    nc.sync.dma_start(
                        out=_dap(out, I * 256 + di * 128 + Jh * 64,
                                 [[8192, 128], [1048576, 4], [1, 64]]),
                        in_=_sap(fin, di * 64, [[128, 4], [1, 64]]))

            gx = {0: gx_load(0), 1: gx_load(1)}
            lu = {0: lup_load(0)}
            gq = {0: stage_qkv(gx.pop(0), wg, False)}
            fins = {}
            for ti in range(64):
                if ti + 2 < 64:
                    gx[ti + 2] = gx_load(ti + 2)
                if ti + 1 < 64:
                    lu[ti + 1] = lup_load(ti + 1)
                    gq[ti + 1] = stage_qkv(gx.pop(ti + 1), wg, False)
                psSts = stage_scores(gq[ti])
                if ti - 1 in fins:
                    g_fin(ti - 1, fins.pop(ti - 1))
                O = stage_pv(gq.pop(ti), psSts)
                fins[ti] = g_proj(O, lu.pop(ti))
                if ti in (4, 8, 12, 16):
                    smooth_strip(1, (ti - 4) // 4)
            g_fin(63, fins.pop(63))
    return nc


def _get_nc():
    if 'nc' not in _RUN_CACHE:
        _RUN_CACHE['nc'] = _build_nc()
    return _RUN_CACHE['nc']


def make_in_maps(inputs):
    import ml_dtypes
    bf16 = ml_dtypes.bfloat16
    x = np.asarray(inputs['x'], dtype=np.float32)
    Wqkv = np.asarray(inputs['Wqkv'], dtype=np.float32)
    bqkv = np.asarray(inputs['bqkv'], dtype=np.float32)
    Wproj = np.asarray(inputs['Wproj'], dtype=np.float32)
    bproj = np.asarray(inputs['bproj'], dtype=np.float32)

    bq, bv = bqkv[:512], bqkv[1024:]
    wqkv_g = np.ascontiguousarray(Wqkv.astype(bf16)).reshape(-1)
    wqkv_l = np.ascontiguousarray((0.25 * Wqkv).astype(bf16)).reshape(-1)
    wproj = np.ascontiguousarray(Wproj.astype(bf16)).reshape(-1)
    bqmat = np.zeros((512, 8), np.float32)
    bqmat[np.arange(512), np.arange(512) // 64] = bq
    bqmat = bqmat.astype(bf16).reshape(-1)
    p = np.arange(128)
    maskb = np.zeros((32, 128), np.float32)
    maskb[(p % 64) // 2, p] = MSQ
    maskb = maskb.astype(bf16).reshape(-1)
    obias = (bproj + bv @ Wproj).astype(bf16)
    bvv = bv.astype(bf16)

    in_maps = []
    shards = []
    for b in range(B):
        for half in range(2):
            shards.append((b, half))
            xs = np.ascontiguousarray(
                x[b, 256 * half:256 * (half + 1)]).reshape(-1).astype(bf16)
            in_maps.append({
                "xs": xs, "wqkv_g": wqkv_g, "wqkv_l": wqkv_l,
                "wproj": wproj, "bqmat": bqmat, "maskb": maskb,
                "obias": obias, "bvv": bvv,
            })
    return in_maps, shards


def kernel(**inputs):
    from concourse.bass_utils import run_bass_kernel_spmd
    nc = _get_nc()
    in_maps, shards = make_in_maps(inputs)
    r = run_bass_kernel_spmd(nc, in_maps, core_ids=list(range(8)))
    _RUN_CACHE['last_result'] = r
    full = np.empty((B, C, H, W), dtype=np.float32)
    for (b, half), res in zip(shards, r.results):
        full[b, :, 64 * half:64 * (half + 1), :] = res["out"]
    return full
